# revision 39
# baseline (speedup 1.0000x reference)
"""Multi-head causal attention with RoPE on 8 TRN2 NeuronCores.

Tensor-parallel over heads: core c computes heads (2c, 2c+1). Single
fused region keeps the PE gapless (sustained PE row rate is ~0.5ns/row
and drops after any idle gap, so the schedule aims for zero PE stalls):

  x-chunk pairs (1024 tokens) are projected to Q^T/K^T (RoPE) and V —
  all-bf16 operands, Q/K/V resident in SBUF — and causal-attention
  sections (batch, head, 512-query chunk) are emitted as soon as their
  chunks exist, so attention's act-engine handoffs are filled with
  projection matmuls and vice versa. Per-(batch, head) half-AllToAlls
  (context head-shard -> token-shard) issue mid-stream as each head's
  context completes and hide under later compute.

  DMA discipline: every bulk load is a contiguous [128, N] 2D slice of
  a HOST-PREPACKED dram tensor (xt2 / wqkv / woG), so each issue costs
  ~600ns on the issuing engine (128 descriptors) regardless of size —
  3D access patterns cost ~5ns/descriptor on the issuing engine and
  were the previous bottleneck. All loads ride the sync ring (an issue
  with an unsatisfied wait blocks everything behind it on that engine's
  queue, so the compute queues carry no loads). A warmup matmul chain
  on a memset tile covers the ~7us engine preamble and pre-ramps the
  PE p-state before the first projection matmul.

  Tail: Wo + b1's deferred sections. hl=1 sections run first so the
  last AllToAll fires early; Wo-b0 m-tiles are interleaved between
  sections as PE fill; Wo-b1 is one PSUM accumulation pass per m-tile
  (odd kbs from the early A2A, then even kbs) with no SBUF staging.

Softmax: scores^T = K^T_blk^T @ Q^T per 128-key block, exp on the Act
engine (no max-subtraction; scores are O(1)), denominators via DVE
elementwise key-sums (two parallel f32 chains) + one gpsimd
partition-all-reduce per section, reciprocal via the fast DVE
approximation. The context PSUM bank is released immediately after the
last ctx matmul via an Act-engine copy of the unnormalized context;
normalization then happens SBUF->SBUF off the PE critical path.

Host does layout prep (x transpose + bf16 cast + packing, RoPE tables,
causal mask tiles) and final unshard (interleave per-core slices).
"""
import ml_dtypes
import numpy as np

import concourse.bass as bass  # noqa: F401  (engine namespaces live on nc)
import concourse.bass_isa as bass_isa
import concourse.mybir as mybir
import concourse.tile as tile
from concourse import bacc
from concourse import bass_utils

B, T, DM, H, D = 2, 2048, 2048, 16, 128
NCORES = 8
HPC = H // NCORES        # heads per core
DLOC = HPC * D           # local head width (256)
BT = B * T               # 4096 token rows
P = 128
TCH = 512                # query chunk
XCH = 1024               # x-chunk pair width
NKB = DM // P            # 16 contraction blocks
NTB = T // P             # 16 token blocks per batch
TSL = T // NCORES        # 256-token output slice per core per batch
SCALE = 1.0 / float(np.sqrt(D))
F32 = mybir.dt.float32
BF16 = mybir.dt.bfloat16
MUL = mybir.AluOpType.mult
ADD = mybir.AluOpType.add
COPY = mybir.ActivationFunctionType.Copy
EXP = mybir.ActivationFunctionType.Exp

_nc_cache = None


def _build():
    nc = bacc.Bacc("TRN2", target_bir_lowering=False, debug=False,
                   num_devices=NCORES)
    # host-packed layouts: every slice the kernel loads is a contiguous
    # [128, N] 2D block (cheap descriptors)
    xt2 = nc.dram_tensor("xt2", [P, 4 * NKB * XCH], BF16, kind="ExternalInput")
    wqk = nc.dram_tensor("wqk", [P, NKB * 2 * DLOC], BF16,
                         kind="ExternalInput")
    wvp = nc.dram_tensor("wvp", [P, NKB * DLOC], BF16, kind="ExternalInput")
    woG = nc.dram_tensor("woG", [P, 8 * NKB * DLOC], BF16, kind="ExternalInput")
    cf = nc.dram_tensor("cf", [P, T], F32, kind="ExternalInput")
    sf = nc.dram_tensor("sf", [P, T], F32, kind="ExternalInput")
    cm = nc.dram_tensor("cm", [P, 4 * TCH], BF16, kind="ExternalInput")
    # out^T slice: [out_cols, b0 slice | b1 slice]
    outT = nc.dram_tensor("out", [DM, B * TSL], F32, kind="ExternalOutput")

    with tile.TileContext(nc) as tc:
        with tc.tile_pool(name="dram", bufs=1, space="DRAM") as dpool, \
             tc.tile_pool(name="const", bufs=1) as cpool, \
             tc.tile_pool(name="qkv", bufs=1) as qpool:
            # per-(batch, local-head) A2A halves: each fires as soon as that
            # head's context is complete, so the tail Wo can start on the
            # gathered half while the other half is still in flight
            ctxH_d = [[dpool.tile([NCORES * P, TSL], BF16, name=f"ctxH{b}{hl}")
                       for hl in range(HPC)] for b in range(B)]
            gouth_d = [[dpool.tile([NCORES * P, TSL], BF16, name=f"gouth{b}{hl}")
                        for hl in range(HPC)] for b in range(B)]
            bar_in = dpool.tile([8, 4], F32)
            bar_out = dpool.tile([64, 4], F32, addr_space="Shared")

            qT_sb = [qpool.tile([P, HPC, T], BF16, name=f"qT{b}") for b in range(B)]
            kT_sb = [qpool.tile([P, HPC, T], BF16, name=f"kT{b}") for b in range(B)]
            v_sb = [qpool.tile([P, NTB, DLOC], BF16, name=f"v{b}") for b in range(B)]

            cm_s = cpool.tile([P, 4 * TCH], BF16)
            wu_s = cpool.tile([P, TCH], BF16)

            # start-skew absorber: cores align here while projections run
            nc.sync.dma_start(bar_in[:], cf.ap()[0:8, 0:4])
            nc.gpsimd.collective_compute(
                "AllGather", mybir.AluOpType.bypass,
                replica_groups=[list(range(NCORES))],
                ins=[bar_in[:].opt()], outs=[bar_out[:].opt()])

            # ---------- fused projections + attention ----------
            with tc.tile_pool(name="p2", bufs=2) as pool2, \
                 tc.tile_pool(name="p2t", bufs=20) as ppool, \
                 tc.tile_pool(name="ps_s", bufs=3, space="PSUM") as ps_sp, \
                 tc.tile_pool(name="ps_acc", bufs=1, space="PSUM") as ps_accp:

                # warmup: memset a tile and run throwaway matmuls so the PE
                # is busy (and p-state ramping) while the first real loads
                # land behind the ~7us engine preamble
                nc.vector.memset(wu_s[:], 0.0)
                for _ in range(10):
                    ps_wu = ps_sp.tile([P, TCH], F32, tag="s")
                    nc.tensor.matmul(ps_wu[:], wu_s[:, 0:P], wu_s[:],
                                     start=True, stop=True)

                def attn_section(b, hl, cq):
                    """One (batch, head, 512-query-chunk) causal-attention
                    section; needs x-chunks <= cq of batch b projected.
                    Diagonal key blocks drop their fully-masked left columns
                    (widths 512/384/256/128)."""
                    nblk = 4 * cq + 4
                    q0 = cq * TCH
                    ps_ctx = ps_accp.tile([P, TCH], F32, tag="ctx")
                    # f32 accumulators (bf16 would drop the tail terms of
                    # long sums); two parallel chains halve DVE latency
                    acc0 = pool2.tile([P, TCH], F32, tag="acc0")
                    acc1 = pool2.tile([P, TCH], F32, tag="acc1")
                    for j in range(nblk):
                        vmask = j - 4 * cq
                        off = vmask * P if vmask > 0 else 0
                        sk = (off > 0)
                        ps_sc = ps_sp.tile([P, TCH], F32, tag="s")
                        nc.tensor.matmul(
                            ps_sc[:, off:], kT_sb[b][:, hl, j * P:(j + 1) * P],
                            qT_sb[b][:, hl, q0 + off:q0 + TCH],
                            start=True, stop=True)
                        pT = ppool.tile([P, TCH], BF16, tag="pT")
                        nc.scalar.activation(pT[:, off:], ps_sc[:, off:],
                                             EXP, scale=SCALE)
                        if vmask >= 0:
                            nc.vector.tensor_tensor(
                                pT[:, off:], pT[:, off:],
                                cm_s[:, vmask * TCH + off:(vmask + 1) * TCH],
                                MUL)
                        nc.tensor.matmul(
                            ps_ctx[:, off:], v_sb[b][:, j, hl * D:(hl + 1) * D],
                            pT[:, off:], start=(j == 0), stop=(j == nblk - 1),
                            skip_group_check=sk)
                        # denominator: elementwise key-sum on the DVE — no
                        # PE ones-matmul at all
                        acc = acc0 if j % 2 == 0 else acc1
                        if j < 2:
                            if off > 0:
                                nc.vector.memset(acc[:, 0:off], 0.0)
                            nc.vector.tensor_copy(acc[:, off:], pT[:, off:])
                        else:
                            nc.vector.tensor_tensor(
                                acc[:, off:], acc[:, off:], pT[:, off:], ADD)
                    # release the ctx PSUM bank right away: unnormalized
                    # copy on Act; normalize SBUF->SBUF once the
                    # denominator is in (off the PE critical path)
                    ctx_u = pool2.tile([P, TCH], BF16, tag="ctx_u")
                    nc.scalar.activation(ctx_u[:], ps_ctx[:], COPY)
                    nc.vector.tensor_tensor(acc0[:], acc0[:], acc1[:], ADD)
                    # all partitions get the key-total in one gpsimd
                    # all-reduce (also kills the partition-broadcast)
                    ar = pool2.tile([P, TCH], F32, tag="ar")
                    nc.gpsimd.partition_all_reduce(
                        ar[:], acc0[:], channels=P,
                        reduce_op=bass_isa.ReduceOp.add)
                    bc_s = pool2.tile([P, TCH], F32, tag="bc_s")
                    nc.vector.reciprocal_approx_fast(bc_s[:], ar[:])
                    ctx_s = pool2.tile([P, TCH], BF16, tag="ctx")
                    nc.vector.tensor_tensor(ctx_s[:], ctx_u[:], bc_s[:], MUL)
                    nc.sync.dma_start(
                        ctxH_d[b][hl]
                        .rearrange("(r p) n -> p r n", p=P)[:, 2 * cq:2 * cq + 2],
                        ctx_s.rearrange("p (r n) -> p r n", r=2))

                projpools = tc.tile_pool(name="p1w", bufs=1), \
                    tc.tile_pool(name="p1cf", bufs=1), \
                    tc.tile_pool(name="p1x", bufs=2), \
                    tc.tile_pool(name="p1", bufs=2), \
                    tc.tile_pool(name="ps1", bufs=3, space="PSUM"), \
                    tc.tile_pool(name="ps1v", bufs=1, space="PSUM")
                wpool = projpools[0].__enter__()
                cfpool = projpools[1].__enter__()
                xpool = projpools[2].__enter__()
                pool = projpools[3].__enter__()
                ps1 = projpools[4].__enter__()
                ps1v = projpools[5].__enter__()
                wsb = wpool.tile([P, NKB, 2 * DLOC], BF16)
                wv_s = wpool.tile([P, NKB, DLOC], BF16)
                cf_s = cfpool.tile([P, T], F32)
                sf_s = cfpool.tile([P, T], F32)

                for ip in range(BT // XCH):     # 4 chunk-pairs
                    bb, icp = ip // 2, ip % 2
                    xt_t = xpool.tile([P, NKB, XCH], BF16, tag="xt")

                    def xt_load(kp, eng):
                        c0 = (ip * NKB + 2 * kp) * XCH
                        eng.dma_start(
                            xt_t[:, 2 * kp:2 * kp + 2],
                            xt2.ap()[:, c0:c0 + 2 * XCH]
                            .rearrange("p (k n) -> p k n", k=2))

                    # ip0/ip1 loads are wait-free (fresh tiles), so they
                    # may ride the scalar ring too — no exp runs before
                    # them that they could block; ip2/ip3 reuse xt buffers
                    # (WAR waits) and must stay on sync
                    if ip == 0:
                        # weight/x kb-pairs interleaved in consumption
                        # order across both rings; RoPE tables, V weights
                        # and mask tiles slot in at their first-use
                        # deadlines
                        for kp in range(NKB // 2):
                            c0 = 2 * kp * 2 * DLOC
                            nc.scalar.dma_start(
                                wsb[:, 2 * kp:2 * kp + 2],
                                wqk.ap()[:, c0:c0 + 4 * DLOC]
                                .rearrange("p (k n) -> p k n", k=2))
                            xt_load(kp, nc.sync)
                            if kp == 2:
                                nc.sync.dma_start(cf_s[:], cf.ap())
                            if kp == 4:
                                nc.scalar.dma_start(sf_s[:], sf.ap())
                            if kp == 6:
                                nc.scalar.dma_start(
                                    wv_s[:, 0:8],
                                    wvp.ap()[:, 0:8 * DLOC]
                                    .rearrange("p (k n) -> p k n", k=8))
                        nc.scalar.dma_start(
                            wv_s[:, 8:16],
                            wvp.ap()[:, 8 * DLOC:16 * DLOC]
                            .rearrange("p (k n) -> p k n", k=8))
                        nc.scalar.dma_start(cm_s[:], cm.ap())
                    elif ip == 1:
                        for kp in range(NKB // 2):
                            xt_load(kp, nc.sync if kp % 2 == 0 else nc.scalar)
                    else:
                        for kp in range(NKB // 2):
                            xt_load(kp, nc.sync)
                    # two query-chunk columns per stationary pass
                    for w0, dst in ((0, qT_sb), (DLOC, kT_sb)):
                        for m in range(HPC):
                            psa = ps1.tile([P, TCH], F32, tag="qk")
                            psb = ps1.tile([P, TCH], F32, tag="qk")
                            for kb in range(NKB):
                                st, sp = (kb == 0), (kb == NKB - 1)
                                w_blk = wsb[:, kb, w0 + m * P:w0 + (m + 1) * P]
                                nc.tensor.matmul(psa[:], w_blk,
                                                 xt_t[:, kb, 0:TCH],
                                                 start=st, stop=sp)
                                nc.tensor.matmul(psb[:], w_blk,
                                                 xt_t[:, kb, TCH:XCH],
                                                 start=st, stop=sp)
                            for half, ps in ((0, psa), (1, psb)):
                                ic = 2 * icp + half
                                c0 = ic * TCH
                                cs = cf_s[:, c0:c0 + TCH]
                                sn = sf_s[:, c0:c0 + TCH]
                                tmp = pool.tile([P, TCH], F32, tag="tmp")
                                tmp2 = pool.tile([P, TCH], F32, tag="tmp2")
                                nc.vector.tensor_tensor(tmp[0:64], ps[64:128],
                                                        sn[0:64], MUL)
                                nc.vector.tensor_tensor(tmp[64:128], ps[0:64],
                                                        sn[64:128], MUL)
                                nc.vector.tensor_tensor(tmp2[:], ps[:], cs, MUL)
                                nc.vector.tensor_tensor(
                                    dst[bb][:, m, c0:c0 + TCH],
                                    tmp2[:], tmp[:], ADD)
                    for tb in range(XCH // P):
                        psv = ps1v.tile([P, DLOC], F32, tag="v")
                        for kb in range(NKB):
                            nc.tensor.matmul(
                                psv[:], xt_t[:, kb, tb * P:(tb + 1) * P],
                                wv_s[:, kb],
                                start=(kb == 0), stop=(kb == NKB - 1))
                        nc.scalar.activation(
                            v_sb[bb][:, icp * (XCH // P) + tb, :], psv[:],
                            COPY)
                    # attention sections whose query chunks now exist; all
                    # of b1's sections are deferred past the projection
                    # pools so they interleave with Wo-b0 instead (at
                    # ip2/ip3's end there is no projection fill left)
                    if ip == 0:
                        for cq in (0, 1):
                            for hl in range(HPC):
                                attn_section(bb, hl, cq)
                    elif ip == 1:
                        for hl in range(HPC):
                            for cq in (2, 3):
                                attn_section(bb, hl, cq)
                            nc.gpsimd.collective_compute(
                                "AllToAll", mybir.AluOpType.bypass,
                                replica_groups=[list(range(NCORES))],
                                ins=[ctxH_d[0][hl][:].opt()],
                                outs=[gouth_d[0][hl][:].opt()])

                for p in reversed(projpools):
                    p.__exit__(None, None, None)

                # ---------- tail: last sections + output projection ----------
                with tc.tile_pool(name="p3w", bufs=1) as wpool3, \
                     tc.tile_pool(name="p3", bufs=2) as pool3, \
                     tc.tile_pool(name="ps3", bufs=4, space="PSUM") as ps3:
                    # wo_s[:, e] holds out-columns [e*256, (e+1)*256) for
                    # all kbs — m-tile m lives in eighth m//2
                    wo_s = wpool3.tile([P, 8, NKB, DLOC], BF16)
                    g_t = [wpool3.tile([P, NKB, TSL], BF16, name=f"g{b}")
                           for b in range(B)]

                    def gather_load(b, hl, engs):
                        # per-r 2D gathers (128 descriptors each); gouth
                        # block r holds global head 2r+hl -> kb slot 2r+hl
                        for r in range(NCORES):
                            engs[r % len(engs)].dma_start(
                                g_t[b][:, 2 * r + hl],
                                gouth_d[b][hl]
                                .rearrange("(r p) n -> r p n", p=P)[r])

                    # Wo streams in column-eighths (contiguous 2D slices of
                    # the host-packed woG) so m-tiles unblock progressively;
                    # b0 gathers (A2As fired back at ip1) slot in after the
                    # first two eighths — everything lands just before its
                    # first consumer
                    def wo_load(e):
                        nc.sync.dma_start(
                            wo_s[:, e],
                            woG.ap()[:, e * NKB * DLOC:(e + 1) * NKB * DLOC]
                            .rearrange("p (k n) -> p k n", k=NKB))

                    wo_load(0)
                    wo_load(1)
                    for hl in range(HPC):
                        gather_load(0, hl, (nc.sync,))
                    for e in range(2, 8):
                        wo_load(e)

                    def wo_b0(mlist):
                        for m in mlist:
                            pso = ps3.tile([P, TSL], F32, tag="o")
                            for kb in range(NKB):
                                nc.tensor.matmul(
                                    pso[:],
                                    wo_s[:, m // 2, kb,
                                         (m % 2) * P:(m % 2 + 1) * P],
                                    g_t[0][:, kb],
                                    start=(kb == 0), stop=(kb == NKB - 1))
                            o_s = pool3.tile([P, TSL], F32, tag="o_s")
                            nc.vector.tensor_copy(o_s[:], pso[:])
                            # all tail DMA rides sync: the scalar queue
                            # must stay pure Act compute (exps gate the
                            # ctx matmuls)
                            nc.sync.dma_start(
                                outT.ap()[m * P:(m + 1) * P, 0:TSL], o_s[:])

                    # hl=1 first: its A2A feeds the first half of the final
                    # Wo-b1 accumulation, so firing it early hides the
                    # collective + peer skew under Wo-b0; a couple of Wo-b0
                    # m-tiles sit between sections as PE fill, but both
                    # A2As stay early so the b1 accumulation never waits
                    attn_section(1, 1, 0)
                    attn_section(1, 1, 1)
                    attn_section(1, 1, 2)
                    wo_b0([0, 1])
                    attn_section(1, 1, 3)
                    nc.gpsimd.collective_compute(
                        "AllToAll", mybir.AluOpType.bypass,
                        replica_groups=[list(range(NCORES))],
                        ins=[ctxH_d[1][1][:].opt()],
                        outs=[gouth_d[1][1][:].opt()])
                    attn_section(1, 0, 0)
                    wo_b0([2, 3])
                    attn_section(1, 0, 1)
                    attn_section(1, 0, 2)
                    wo_b0([4, 5])
                    attn_section(1, 0, 3)
                    nc.gpsimd.collective_compute(
                        "AllToAll", mybir.AluOpType.bypass,
                        replica_groups=[list(range(NCORES))],
                        ins=[ctxH_d[1][0][:].opt()],
                        outs=[gouth_d[1][0][:].opt()])
                    # gathers only now: every ctx scatter and exp is
                    # already queued ahead of them, so their parked A2A
                    # waits can no longer delay a collective's input; the
                    # last gather splits across both rings to halve its
                    # post-A2A issue latency
                    gather_load(1, 1, (nc.sync,))
                    gather_load(1, 0, (nc.sync, nc.scalar))
                    wo_b0([6, 7, 8, 9, 10, 11, 12, 13, 14, 15])
                    # Wo-b1: one PSUM pass per m-tile — odd kbs (early A2A)
                    # first, even kbs accumulate on top once their gather
                    # lands; no SBUF staging, no DVE adds
                    for m in range(DM // P):
                        psA = ps3.tile([P, TSL], F32, tag="o")
                        for i in range(NCORES):
                            nc.tensor.matmul(
                                psA[:],
                                wo_s[:, m // 2, 2 * i + 1,
                                     (m % 2) * P:(m % 2 + 1) * P],
                                g_t[1][:, 2 * i + 1],
                                start=(i == 0), stop=False)
                        for i in range(NCORES):
                            nc.tensor.matmul(
                                psA[:],
                                wo_s[:, m // 2, 2 * i,
                                     (m % 2) * P:(m % 2 + 1) * P],
                                g_t[1][:, 2 * i],
                                start=False, stop=(i == NCORES - 1))
                        o_s = pool3.tile([P, TSL], F32, tag="o_s")
                        nc.vector.tensor_copy(o_s[:], psA[:])
                        nc.sync.dma_start(
                            outT.ap()[m * P:(m + 1) * P, TSL:2 * TSL], o_s[:])

    nc.compile()
    return nc


def _prep_inputs(x, cos, sin, Wq, Wk, Wv, Wo):
    x = np.asarray(x, dtype=np.float32)
    cos = np.asarray(cos, dtype=np.float32)
    sin = np.asarray(sin, dtype=np.float32)
    xt = np.ascontiguousarray(x.reshape(BT, DM).T).astype(ml_dtypes.bfloat16)
    # xt2[p, ip*16K + kb*1K + n] = xt[kb*128+p, ip*1024+n]
    xt2 = np.ascontiguousarray(
        xt.reshape(NKB, P, 4, XCH).transpose(1, 2, 0, 3).reshape(P, -1))
    cf = np.empty((P, T), np.float32)
    cf[:64] = cos.T
    cf[64:] = cos.T
    sf = np.empty((P, T), np.float32)
    sf[:64] = -sin.T
    sf[64:] = sin.T
    qq = np.arange(TCH, dtype=np.int64)[None, :]
    rr = np.arange(P, dtype=np.int64)[:, None]
    cm = np.concatenate(
        [(qq >= v * P + rr).astype(np.float32) for v in range(TCH // P)],
        axis=1).astype(ml_dtypes.bfloat16)
    # woG[p, e*4K + kb*256 + c] = Wo[kb*128+p, e*256+c]
    wo16 = np.asarray(Wo, np.float32).astype(ml_dtypes.bfloat16)
    woG = np.ascontiguousarray(
        wo16.reshape(NKB, P, 8, DLOC).transpose(1, 2, 0, 3).reshape(P, -1))
    wq16 = np.asarray(Wq, np.float32).astype(ml_dtypes.bfloat16)
    wk16 = np.asarray(Wk, np.float32).astype(ml_dtypes.bfloat16)
    wv16 = np.asarray(Wv, np.float32).astype(ml_dtypes.bfloat16)
    in_maps = []
    for c in range(NCORES):
        sl = slice(c * DLOC, (c + 1) * DLOC)
        # wqk[p, kb*512 + j]: j in [0,256) wq | [256,512) wk
        wqk = np.ascontiguousarray(
            np.concatenate(
                [wq16[:, sl].reshape(NKB, P, DLOC),
                 wk16[:, sl].reshape(NKB, P, DLOC)],
                axis=2).transpose(1, 0, 2).reshape(P, -1))
        wvp = np.ascontiguousarray(
            wv16[:, sl].reshape(NKB, P, DLOC).transpose(1, 0, 2)
            .reshape(P, -1))
        in_maps.append({
            "xt2": xt2, "cf": cf, "sf": sf, "cm": cm,
            "wqk": wqk, "wvp": wvp, "woG": woG,
        })
    return in_maps


def run(x, mask, cos, sin, Wq, Wk, Wv, Wo, trace=False, trace_cores=None):
    global _nc_cache
    if _nc_cache is None:
        _nc_cache = _build()
    in_maps = _prep_inputs(x, cos, sin, Wq, Wk, Wv, Wo)
    kwargs = {"trace_cores": trace_cores} if trace_cores else {}
    res = bass_utils.run_bass_kernel_spmd(
        _nc_cache, in_maps, core_ids=list(range(NCORES)), trace=trace, **kwargs)
    out = np.empty((B, T, DM), np.float32)
    for c in range(NCORES):
        o = res.results[c]["out"]  # [DM, B*TSL]
        for b in range(B):
            out[b, c * TSL:(c + 1) * TSL, :] = o[:, b * TSL:(b + 1) * TSL].T
    return out, res


def kernel(x, mask, cos, sin, Wq, Wk, Wv, Wo):
    out, _ = run(x, mask, cos, sin, Wq, Wk, Wv, Wo, trace=False)
    return out


# revision 42
# speedup vs baseline: 1.0069x; 1.0069x over previous
"""Multi-head causal attention with RoPE on 8 TRN2 NeuronCores.

Tensor-parallel over heads: core c computes heads (2c, 2c+1). Single
fused region keeps the PE gapless (sustained PE row rate is ~0.5ns/row
and drops after any idle gap, so the schedule aims for zero PE stalls):

  x-chunk pairs (1024 tokens) are projected to Q^T/K^T (RoPE) and V —
  all-bf16 operands, Q/K/V resident in SBUF — and causal-attention
  sections (batch, head, 512-query chunk) are emitted as soon as their
  chunks exist, so attention's act-engine handoffs are filled with
  projection matmuls and vice versa. Per-(batch, head) half-AllToAlls
  (context head-shard -> token-shard) issue mid-stream as each head's
  context completes and hide under later compute.

  DMA discipline: every bulk load is a contiguous [128, N] 2D slice of
  a HOST-PREPACKED dram tensor (xt2 / wqkv / woG), so each issue costs
  ~600ns on the issuing engine (128 descriptors) regardless of size —
  3D access patterns cost ~5ns/descriptor on the issuing engine and
  were the previous bottleneck. All loads ride the sync ring (an issue
  with an unsatisfied wait blocks everything behind it on that engine's
  queue, so the compute queues carry no loads). A warmup matmul chain
  on a memset tile covers the ~7us engine preamble and pre-ramps the
  PE p-state before the first projection matmul.

  Tail: Wo + b1's deferred sections. hl=1 sections run first so the
  last AllToAll fires early; Wo-b0 m-tiles are interleaved between
  sections as PE fill; Wo-b1 is one PSUM accumulation pass per m-tile
  (odd kbs from the early A2A, then even kbs) with no SBUF staging.

Softmax: scores^T = K^T_blk^T @ Q^T per 128-key block, exp on the Act
engine (no max-subtraction; scores are O(1)), denominators via DVE
elementwise key-sums (two parallel f32 chains) + one gpsimd
partition-all-reduce per section, reciprocal via the fast DVE
approximation. The context PSUM bank is released immediately after the
last ctx matmul via an Act-engine copy of the unnormalized context;
normalization then happens SBUF->SBUF off the PE critical path.

Host does layout prep (x transpose + bf16 cast + packing, RoPE tables,
causal mask tiles) and final unshard (interleave per-core slices).
"""
import ml_dtypes
import numpy as np

import concourse.bass as bass  # noqa: F401  (engine namespaces live on nc)
import concourse.bass_isa as bass_isa
import concourse.mybir as mybir
import concourse.tile as tile
from concourse import bacc
from concourse import bass_utils

B, T, DM, H, D = 2, 2048, 2048, 16, 128
NCORES = 8
HPC = H // NCORES        # heads per core
DLOC = HPC * D           # local head width (256)
BT = B * T               # 4096 token rows
P = 128
TCH = 512                # query chunk
XCH = 1024               # x-chunk pair width
NKB = DM // P            # 16 contraction blocks
NTB = T // P             # 16 token blocks per batch
TSL = T // NCORES        # 256-token output slice per core per batch
SCALE = 1.0 / float(np.sqrt(D))
F32 = mybir.dt.float32
BF16 = mybir.dt.bfloat16
MUL = mybir.AluOpType.mult
ADD = mybir.AluOpType.add
COPY = mybir.ActivationFunctionType.Copy
EXP = mybir.ActivationFunctionType.Exp

_nc_cache = None


def _build():
    nc = bacc.Bacc("TRN2", target_bir_lowering=False, debug=False,
                   num_devices=NCORES)
    # host-packed layouts: every slice the kernel loads is a contiguous
    # [128, N] 2D block (cheap descriptors)
    xt2 = nc.dram_tensor("xt2", [P, 4 * NKB * XCH], BF16, kind="ExternalInput")
    wqk = nc.dram_tensor("wqk", [P, NKB * 2 * DLOC], BF16,
                         kind="ExternalInput")
    wvp = nc.dram_tensor("wvp", [P, NKB * DLOC], BF16, kind="ExternalInput")
    woG = nc.dram_tensor("woG", [P, 8 * NKB * DLOC], BF16, kind="ExternalInput")
    cf = nc.dram_tensor("cf", [P, T], F32, kind="ExternalInput")
    sf = nc.dram_tensor("sf", [P, T], F32, kind="ExternalInput")
    cm = nc.dram_tensor("cm", [P, 4 * TCH], BF16, kind="ExternalInput")
    # out^T slice: [out_cols, b0 slice | b1 slice]
    outT = nc.dram_tensor("out", [DM, B * TSL], F32, kind="ExternalOutput")

    with tile.TileContext(nc) as tc:
        with tc.tile_pool(name="dram", bufs=1, space="DRAM") as dpool, \
             tc.tile_pool(name="const", bufs=1) as cpool, \
             tc.tile_pool(name="qkv", bufs=1) as qpool:
            # per-(batch, local-head) A2A halves: each fires as soon as that
            # head's context is complete, so the tail Wo can start on the
            # gathered half while the other half is still in flight
            ctxH_d = [[dpool.tile([NCORES * P, TSL], BF16, name=f"ctxH{b}{hl}")
                       for hl in range(HPC)] for b in range(B)]
            gouth_d = [[dpool.tile([NCORES * P, TSL], BF16, name=f"gouth{b}{hl}")
                        for hl in range(HPC)] for b in range(B)]
            bar_in = dpool.tile([8, 4], F32)
            bar_out = dpool.tile([64, 4], F32, addr_space="Shared")

            qT_sb = [qpool.tile([P, HPC, T], BF16, name=f"qT{b}") for b in range(B)]
            kT_sb = [qpool.tile([P, HPC, T], BF16, name=f"kT{b}") for b in range(B)]
            v_sb = [qpool.tile([P, NTB, DLOC], BF16, name=f"v{b}") for b in range(B)]

            cm_s = cpool.tile([P, 4 * TCH], BF16)
            wu_s = cpool.tile([P, TCH], BF16)

            # start-skew absorber: cores align here while projections run
            nc.sync.dma_start(bar_in[:], cf.ap()[0:8, 0:4])
            nc.gpsimd.collective_compute(
                "AllGather", mybir.AluOpType.bypass,
                replica_groups=[list(range(NCORES))],
                ins=[bar_in[:].opt()], outs=[bar_out[:].opt()])

            # ---------- fused projections + attention ----------
            with tc.tile_pool(name="p2", bufs=2) as pool2, \
                 tc.tile_pool(name="p2t", bufs=20) as ppool, \
                 tc.tile_pool(name="ps_s", bufs=3, space="PSUM") as ps_sp, \
                 tc.tile_pool(name="ps_acc", bufs=1, space="PSUM") as ps_accp:

                # warmup: memset a tile and run throwaway matmuls so the PE
                # is busy (and p-state ramping) while the first real loads
                # land behind the ~7us engine preamble
                nc.vector.memset(wu_s[:], 0.0)
                for _ in range(10):
                    ps_wu = ps_sp.tile([P, TCH], F32, tag="s")
                    nc.tensor.matmul(ps_wu[:], wu_s[:, 0:P], wu_s[:],
                                     start=True, stop=True)

                def attn_section(b, hl, cq):
                    """One (batch, head, 512-query-chunk) causal-attention
                    section; needs x-chunks <= cq of batch b projected.
                    Diagonal key blocks drop their fully-masked left columns
                    (widths 512/384/256/128)."""
                    nblk = 4 * cq + 4
                    q0 = cq * TCH
                    ps_ctx = ps_accp.tile([P, TCH], F32, tag="ctx")
                    # f32 accumulators (bf16 would drop the tail terms of
                    # long sums); two parallel chains halve DVE latency
                    acc0 = pool2.tile([P, TCH], F32, tag="acc0")
                    acc1 = pool2.tile([P, TCH], F32, tag="acc1")
                    for j in range(nblk):
                        vmask = j - 4 * cq
                        off = vmask * P if vmask > 0 else 0
                        sk = (off > 0)
                        ps_sc = ps_sp.tile([P, TCH], F32, tag="s")
                        nc.tensor.matmul(
                            ps_sc[:, off:], kT_sb[b][:, hl, j * P:(j + 1) * P],
                            qT_sb[b][:, hl, q0 + off:q0 + TCH],
                            start=True, stop=True)
                        pT = ppool.tile([P, TCH], BF16, tag="pT")
                        nc.scalar.activation(pT[:, off:], ps_sc[:, off:],
                                             EXP, scale=SCALE)
                        if vmask >= 0:
                            nc.vector.tensor_tensor(
                                pT[:, off:], pT[:, off:],
                                cm_s[:, vmask * TCH + off:(vmask + 1) * TCH],
                                MUL)
                        nc.tensor.matmul(
                            ps_ctx[:, off:], v_sb[b][:, j, hl * D:(hl + 1) * D],
                            pT[:, off:], start=(j == 0), stop=(j == nblk - 1),
                            skip_group_check=sk)
                        # denominator: elementwise key-sum on the DVE — no
                        # PE ones-matmul at all
                        acc = acc0 if j % 2 == 0 else acc1
                        if j < 2:
                            if off > 0:
                                nc.vector.memset(acc[:, 0:off], 0.0)
                            nc.vector.tensor_copy(acc[:, off:], pT[:, off:])
                        else:
                            nc.vector.tensor_tensor(
                                acc[:, off:], acc[:, off:], pT[:, off:], ADD)
                    # release the ctx PSUM bank right away: unnormalized
                    # copy on Act; normalize SBUF->SBUF once the
                    # denominator is in (off the PE critical path)
                    ctx_u = pool2.tile([P, TCH], BF16, tag="ctx_u")
                    nc.scalar.activation(ctx_u[:], ps_ctx[:], COPY)
                    nc.vector.tensor_tensor(acc0[:], acc0[:], acc1[:], ADD)
                    # all partitions get the key-total in one gpsimd
                    # all-reduce (also kills the partition-broadcast)
                    ar = pool2.tile([P, TCH], F32, tag="ar")
                    nc.gpsimd.partition_all_reduce(
                        ar[:], acc0[:], channels=P,
                        reduce_op=bass_isa.ReduceOp.add)
                    bc_s = pool2.tile([P, TCH], F32, tag="bc_s")
                    nc.vector.reciprocal_approx_fast(bc_s[:], ar[:])
                    ctx_s = pool2.tile([P, TCH], BF16, tag="ctx")
                    nc.vector.tensor_tensor(ctx_s[:], ctx_u[:], bc_s[:], MUL)
                    nc.sync.dma_start(
                        ctxH_d[b][hl]
                        .rearrange("(r p) n -> p r n", p=P)[:, 2 * cq:2 * cq + 2],
                        ctx_s.rearrange("p (r n) -> p r n", r=2))

                projpools = tc.tile_pool(name="p1w", bufs=1), \
                    tc.tile_pool(name="p1cf", bufs=1), \
                    tc.tile_pool(name="p1x", bufs=2), \
                    tc.tile_pool(name="p1", bufs=2), \
                    tc.tile_pool(name="ps1", bufs=3, space="PSUM"), \
                    tc.tile_pool(name="ps1v", bufs=1, space="PSUM")
                wpool = projpools[0].__enter__()
                cfpool = projpools[1].__enter__()
                xpool = projpools[2].__enter__()
                pool = projpools[3].__enter__()
                ps1 = projpools[4].__enter__()
                ps1v = projpools[5].__enter__()
                wsb = wpool.tile([P, NKB, 2 * DLOC], BF16)
                wv_s = wpool.tile([P, NKB, DLOC], BF16)
                cf_s = cfpool.tile([P, T], F32)
                sf_s = cfpool.tile([P, T], F32)

                for ip in range(BT // XCH):     # 4 chunk-pairs
                    bb, icp = ip // 2, ip % 2
                    xt_t = xpool.tile([P, NKB, XCH], BF16, tag="xt")

                    def xt_load(kp, eng):
                        c0 = (ip * NKB + 2 * kp) * XCH
                        eng.dma_start(
                            xt_t[:, 2 * kp:2 * kp + 2],
                            xt2.ap()[:, c0:c0 + 2 * XCH]
                            .rearrange("p (k n) -> p k n", k=2))

                    # ip0/ip1 loads are wait-free (fresh tiles), so they
                    # may ride the scalar ring too — no exp runs before
                    # them that they could block; ip2/ip3 reuse xt buffers
                    # (WAR waits) and must stay on sync
                    if ip == 0:
                        # weight/x kb-pairs interleaved in consumption
                        # order across both rings; RoPE tables, V weights
                        # and mask tiles slot in at their first-use
                        # deadlines
                        for kp in range(NKB // 2):
                            c0 = 2 * kp * 2 * DLOC
                            nc.scalar.dma_start(
                                wsb[:, 2 * kp:2 * kp + 2],
                                wqk.ap()[:, c0:c0 + 4 * DLOC]
                                .rearrange("p (k n) -> p k n", k=2))
                            xt_load(kp, nc.sync)
                            if kp == 2:
                                nc.sync.dma_start(cf_s[:], cf.ap())
                            if kp == 4:
                                nc.scalar.dma_start(sf_s[:], sf.ap())
                            if kp == 6:
                                nc.scalar.dma_start(
                                    wv_s[:, 0:8],
                                    wvp.ap()[:, 0:8 * DLOC]
                                    .rearrange("p (k n) -> p k n", k=8))
                        nc.scalar.dma_start(
                            wv_s[:, 8:16],
                            wvp.ap()[:, 8 * DLOC:16 * DLOC]
                            .rearrange("p (k n) -> p k n", k=8))
                        nc.scalar.dma_start(cm_s[:], cm.ap())
                    elif ip == 1:
                        for kp in range(NKB // 2):
                            xt_load(kp, nc.sync if kp % 2 == 0 else nc.scalar)
                    else:
                        for kp in range(NKB // 2):
                            xt_load(kp, nc.sync)
                    # two query-chunk columns per stationary pass
                    for w0, dst in ((0, qT_sb), (DLOC, kT_sb)):
                        for m in range(HPC):
                            psa = ps1.tile([P, TCH], F32, tag="qk")
                            psb = ps1.tile([P, TCH], F32, tag="qk")
                            for kb in range(NKB):
                                st, sp = (kb == 0), (kb == NKB - 1)
                                w_blk = wsb[:, kb, w0 + m * P:w0 + (m + 1) * P]
                                nc.tensor.matmul(psa[:], w_blk,
                                                 xt_t[:, kb, 0:TCH],
                                                 start=st, stop=sp)
                                nc.tensor.matmul(psb[:], w_blk,
                                                 xt_t[:, kb, TCH:XCH],
                                                 start=st, stop=sp)
                            for half, ps in ((0, psa), (1, psb)):
                                ic = 2 * icp + half
                                c0 = ic * TCH
                                cs = cf_s[:, c0:c0 + TCH]
                                sn = sf_s[:, c0:c0 + TCH]
                                tmp = pool.tile([P, TCH], F32, tag="tmp")
                                tmp2 = pool.tile([P, TCH], F32, tag="tmp2")
                                nc.vector.tensor_tensor(tmp[0:64], ps[64:128],
                                                        sn[0:64], MUL)
                                nc.vector.tensor_tensor(tmp[64:128], ps[0:64],
                                                        sn[64:128], MUL)
                                nc.vector.tensor_tensor(tmp2[:], ps[:], cs, MUL)
                                nc.vector.tensor_tensor(
                                    dst[bb][:, m, c0:c0 + TCH],
                                    tmp2[:], tmp[:], ADD)
                    for tb in range(XCH // P):
                        psv = ps1v.tile([P, DLOC], F32, tag="v")
                        for kb in range(NKB):
                            nc.tensor.matmul(
                                psv[:], xt_t[:, kb, tb * P:(tb + 1) * P],
                                wv_s[:, kb],
                                start=(kb == 0), stop=(kb == NKB - 1))
                        nc.scalar.activation(
                            v_sb[bb][:, icp * (XCH // P) + tb, :], psv[:],
                            COPY)
                    # attention sections whose query chunks now exist; all
                    # of b1's sections are deferred past the projection
                    # pools so they interleave with Wo-b0 instead (at
                    # ip2/ip3's end there is no projection fill left)
                    if ip == 0:
                        for cq in (0, 1):
                            for hl in range(HPC):
                                attn_section(bb, hl, cq)
                    elif ip == 1:
                        for hl in range(HPC):
                            for cq in (2, 3):
                                attn_section(bb, hl, cq)
                            nc.gpsimd.collective_compute(
                                "AllToAll", mybir.AluOpType.bypass,
                                replica_groups=[list(range(NCORES))],
                                ins=[ctxH_d[0][hl][:].opt()],
                                outs=[gouth_d[0][hl][:].opt()])

                for p in reversed(projpools):
                    p.__exit__(None, None, None)

                # ---------- tail: last sections + output projection ----------
                with tc.tile_pool(name="p3w", bufs=1) as wpool3, \
                     tc.tile_pool(name="p3", bufs=2) as pool3, \
                     tc.tile_pool(name="ps3", bufs=4, space="PSUM") as ps3:
                    # wo_s[:, e] holds out-columns [e*256, (e+1)*256) for
                    # all kbs — m-tile m lives in eighth m//2
                    wo_s = wpool3.tile([P, 8, NKB, DLOC], BF16)
                    g_t = [wpool3.tile([P, NKB, TSL], BF16, name=f"g{b}")
                           for b in range(B)]

                    def gather_load(b, hl, engs):
                        # per-r 2D gathers (128 descriptors each); gouth
                        # block r holds global head 2r+hl -> kb slot 2r+hl
                        for r in range(NCORES):
                            engs[r % len(engs)].dma_start(
                                g_t[b][:, 2 * r + hl],
                                gouth_d[b][hl]
                                .rearrange("(r p) n -> r p n", p=P)[r])

                    # Wo streams in column-eighths (contiguous 2D slices of
                    # the host-packed woG) so m-tiles unblock progressively;
                    # b0 gathers (A2As fired back at ip1) slot in after the
                    # first two eighths — everything lands just before its
                    # first consumer
                    def wo_load(e):
                        nc.sync.dma_start(
                            wo_s[:, e],
                            woG.ap()[:, e * NKB * DLOC:(e + 1) * NKB * DLOC]
                            .rearrange("p (k n) -> p k n", k=NKB))

                    wo_load(0)
                    wo_load(1)
                    for hl in range(HPC):
                        gather_load(0, hl, (nc.sync,))
                    for e in range(2, 8):
                        wo_load(e)

                    def wo_b0(mlist, alt=False):
                        for m in mlist:
                            pso = ps3.tile([P, TSL], F32, tag="o")
                            for kb in range(NKB):
                                nc.tensor.matmul(
                                    pso[:],
                                    wo_s[:, m // 2, kb,
                                         (m % 2) * P:(m % 2 + 1) * P],
                                    g_t[0][:, kb],
                                    start=(kb == 0), stop=(kb == NKB - 1))
                            o_s = pool3.tile([P, TSL], F32, tag="o_s")
                            nc.vector.tensor_copy(o_s[:], pso[:])
                            # while exps still run, tail DMA rides sync so
                            # the scalar queue stays pure Act compute; once
                            # sections are done (alt=True) writes alternate
                            # rings so the final drain halves
                            eng = nc.scalar if alt and m % 2 else nc.sync
                            eng.dma_start(
                                outT.ap()[m * P:(m + 1) * P, 0:TSL], o_s[:])

                    # hl=1 first: its A2A feeds the first half of the final
                    # Wo-b1 accumulation, so firing it early hides the
                    # collective + peer skew under Wo-b0; a couple of Wo-b0
                    # m-tiles sit between sections as PE fill, but both
                    # A2As stay early so the b1 accumulation never waits
                    attn_section(1, 1, 0)
                    attn_section(1, 1, 1)
                    attn_section(1, 1, 2)
                    wo_b0([0, 1])
                    attn_section(1, 1, 3)
                    nc.gpsimd.collective_compute(
                        "AllToAll", mybir.AluOpType.bypass,
                        replica_groups=[list(range(NCORES))],
                        ins=[ctxH_d[1][1][:].opt()],
                        outs=[gouth_d[1][1][:].opt()])
                    attn_section(1, 0, 0)
                    wo_b0([2, 3])
                    attn_section(1, 0, 1)
                    attn_section(1, 0, 2)
                    wo_b0([4, 5])
                    attn_section(1, 0, 3)
                    nc.gpsimd.collective_compute(
                        "AllToAll", mybir.AluOpType.bypass,
                        replica_groups=[list(range(NCORES))],
                        ins=[ctxH_d[1][0][:].opt()],
                        outs=[gouth_d[1][0][:].opt()])
                    # gathers only now: every ctx scatter and exp is
                    # already queued ahead of them, so their parked A2A
                    # waits can no longer delay a collective's input; the
                    # last gather splits across both rings to halve its
                    # post-A2A issue latency
                    gather_load(1, 1, (nc.sync,))
                    gather_load(1, 0, (nc.sync, nc.scalar))
                    wo_b0([6, 7, 8, 9, 10, 11, 12, 13, 14, 15], alt=True)
                    # Wo-b1: one PSUM pass per m-tile — odd kbs (early A2A)
                    # first, even kbs accumulate on top once their gather
                    # lands; no SBUF staging, no DVE adds
                    for m in range(DM // P):
                        psA = ps3.tile([P, TSL], F32, tag="o")
                        for i in range(NCORES):
                            nc.tensor.matmul(
                                psA[:],
                                wo_s[:, m // 2, 2 * i + 1,
                                     (m % 2) * P:(m % 2 + 1) * P],
                                g_t[1][:, 2 * i + 1],
                                start=(i == 0), stop=False)
                        for i in range(NCORES):
                            nc.tensor.matmul(
                                psA[:],
                                wo_s[:, m // 2, 2 * i,
                                     (m % 2) * P:(m % 2 + 1) * P],
                                g_t[1][:, 2 * i],
                                start=False, stop=(i == NCORES - 1))
                        o_s = pool3.tile([P, TSL], F32, tag="o_s")
                        nc.vector.tensor_copy(o_s[:], psA[:])
                        eng = nc.sync if m % 2 == 0 else nc.scalar
                        eng.dma_start(
                            outT.ap()[m * P:(m + 1) * P, TSL:2 * TSL], o_s[:])

    nc.compile()
    return nc


def _prep_inputs(x, cos, sin, Wq, Wk, Wv, Wo):
    x = np.asarray(x, dtype=np.float32)
    cos = np.asarray(cos, dtype=np.float32)
    sin = np.asarray(sin, dtype=np.float32)
    xt = np.ascontiguousarray(x.reshape(BT, DM).T).astype(ml_dtypes.bfloat16)
    # xt2[p, ip*16K + kb*1K + n] = xt[kb*128+p, ip*1024+n]
    xt2 = np.ascontiguousarray(
        xt.reshape(NKB, P, 4, XCH).transpose(1, 2, 0, 3).reshape(P, -1))
    cf = np.empty((P, T), np.float32)
    cf[:64] = cos.T
    cf[64:] = cos.T
    sf = np.empty((P, T), np.float32)
    sf[:64] = -sin.T
    sf[64:] = sin.T
    qq = np.arange(TCH, dtype=np.int64)[None, :]
    rr = np.arange(P, dtype=np.int64)[:, None]
    cm = np.concatenate(
        [(qq >= v * P + rr).astype(np.float32) for v in range(TCH // P)],
        axis=1).astype(ml_dtypes.bfloat16)
    # woG[p, e*4K + kb*256 + c] = Wo[kb*128+p, e*256+c]
    wo16 = np.asarray(Wo, np.float32).astype(ml_dtypes.bfloat16)
    woG = np.ascontiguousarray(
        wo16.reshape(NKB, P, 8, DLOC).transpose(1, 2, 0, 3).reshape(P, -1))
    wq16 = np.asarray(Wq, np.float32).astype(ml_dtypes.bfloat16)
    wk16 = np.asarray(Wk, np.float32).astype(ml_dtypes.bfloat16)
    wv16 = np.asarray(Wv, np.float32).astype(ml_dtypes.bfloat16)
    in_maps = []
    for c in range(NCORES):
        sl = slice(c * DLOC, (c + 1) * DLOC)
        # wqk[p, kb*512 + j]: j in [0,256) wq | [256,512) wk
        wqk = np.ascontiguousarray(
            np.concatenate(
                [wq16[:, sl].reshape(NKB, P, DLOC),
                 wk16[:, sl].reshape(NKB, P, DLOC)],
                axis=2).transpose(1, 0, 2).reshape(P, -1))
        wvp = np.ascontiguousarray(
            wv16[:, sl].reshape(NKB, P, DLOC).transpose(1, 0, 2)
            .reshape(P, -1))
        in_maps.append({
            "xt2": xt2, "cf": cf, "sf": sf, "cm": cm,
            "wqk": wqk, "wvp": wvp, "woG": woG,
        })
    return in_maps


def run(x, mask, cos, sin, Wq, Wk, Wv, Wo, trace=False, trace_cores=None):
    global _nc_cache
    if _nc_cache is None:
        _nc_cache = _build()
    in_maps = _prep_inputs(x, cos, sin, Wq, Wk, Wv, Wo)
    kwargs = {"trace_cores": trace_cores} if trace_cores else {}
    res = bass_utils.run_bass_kernel_spmd(
        _nc_cache, in_maps, core_ids=list(range(NCORES)), trace=trace, **kwargs)
    out = np.empty((B, T, DM), np.float32)
    for c in range(NCORES):
        o = res.results[c]["out"]  # [DM, B*TSL]
        for b in range(B):
            out[b, c * TSL:(c + 1) * TSL, :] = o[:, b * TSL:(b + 1) * TSL].T
    return out, res


def kernel(x, mask, cos, sin, Wq, Wk, Wv, Wo):
    out, _ = run(x, mask, cos, sin, Wq, Wk, Wv, Wo, trace=False)
    return out


# revision 43
# speedup vs baseline: 1.1237x; 1.1159x over previous
"""Multi-head causal attention with RoPE on 8 TRN2 NeuronCores.

Tensor-parallel over heads: core c computes heads (2c, 2c+1). Single
fused region keeps the PE gapless (sustained PE row rate is ~0.5ns/row
and drops after any idle gap, so the schedule aims for zero PE stalls):

  x-chunk pairs (1024 tokens) are projected to Q^T/K^T (RoPE) and V —
  all-bf16 operands, Q/K/V resident in SBUF — and causal-attention
  sections (batch, head, 512-query chunk) are emitted as soon as their
  chunks exist, so attention's act-engine handoffs are filled with
  projection matmuls and vice versa. Per-(batch, head) half-AllToAlls
  (context head-shard -> token-shard) issue mid-stream as each head's
  context completes and hide under later compute.

  DMA discipline: every bulk load is a contiguous [128, N] 2D slice of
  a HOST-PREPACKED dram tensor (xt2 / wqkv / woG), so each issue costs
  ~600ns on the issuing engine (128 descriptors) regardless of size —
  3D access patterns cost ~5ns/descriptor on the issuing engine and
  were the previous bottleneck. All loads ride the sync ring (an issue
  with an unsatisfied wait blocks everything behind it on that engine's
  queue, so the compute queues carry no loads). A warmup matmul chain
  on a memset tile covers the ~7us engine preamble and pre-ramps the
  PE p-state before the first projection matmul.

  Tail: Wo + b1's deferred sections. hl=1 sections run first so the
  last AllToAll fires early; Wo-b0 m-tiles are interleaved between
  sections as PE fill; Wo-b1 is one PSUM accumulation pass per m-tile
  (odd kbs from the early A2A, then even kbs) with no SBUF staging.

Softmax: scores^T = K^T_blk^T @ Q^T per 128-key block, exp on the Act
engine (no max-subtraction; scores are O(1)), denominators via DVE
elementwise key-sums (two parallel f32 chains) + one gpsimd
partition-all-reduce per section, reciprocal via the fast DVE
approximation. The context PSUM bank is released immediately after the
last ctx matmul via an Act-engine copy of the unnormalized context;
normalization then happens SBUF->SBUF off the PE critical path.

Host does layout prep (x transpose + bf16 cast + packing, RoPE tables,
causal mask tiles) and final unshard (interleave per-core slices).
"""
import ml_dtypes
import numpy as np

import concourse.bass as bass  # noqa: F401  (engine namespaces live on nc)
import concourse.bass_isa as bass_isa
import concourse.mybir as mybir
import concourse.tile as tile
from concourse import bacc
from concourse import bass_utils

B, T, DM, H, D = 2, 2048, 2048, 16, 128
NCORES = 8
HPC = H // NCORES        # heads per core
DLOC = HPC * D           # local head width (256)
BT = B * T               # 4096 token rows
P = 128
TCH = 512                # query chunk
XCH = 1024               # x-chunk pair width
NKB = DM // P            # 16 contraction blocks
NTB = T // P             # 16 token blocks per batch
TSL = T // NCORES        # 256-token output slice per core per batch
SCALE = 1.0 / float(np.sqrt(D))
F32 = mybir.dt.float32
BF16 = mybir.dt.bfloat16
MUL = mybir.AluOpType.mult
ADD = mybir.AluOpType.add
COPY = mybir.ActivationFunctionType.Copy
EXP = mybir.ActivationFunctionType.Exp

_nc_cache = None


def _build():
    nc = bacc.Bacc("TRN2", target_bir_lowering=False, debug=False,
                   num_devices=NCORES)
    # host-packed layouts: every slice the kernel loads is a contiguous
    # [128, N] 2D block (cheap descriptors)
    xt2 = nc.dram_tensor("xt2", [P, 4 * NKB * XCH], BF16, kind="ExternalInput")
    wqk = nc.dram_tensor("wqk", [P, NKB * 2 * DLOC], BF16,
                         kind="ExternalInput")
    wvp = nc.dram_tensor("wvp", [P, NKB * DLOC], BF16, kind="ExternalInput")
    woG = nc.dram_tensor("woG", [P, 8 * NKB * DLOC], BF16, kind="ExternalInput")
    cf = nc.dram_tensor("cf", [P, T], F32, kind="ExternalInput")
    sf = nc.dram_tensor("sf", [P, T], F32, kind="ExternalInput")
    cm = nc.dram_tensor("cm", [P, 4 * TCH], BF16, kind="ExternalInput")
    # out^T slice: [out_cols, b0 slice | b1 slice]
    outT = nc.dram_tensor("out", [DM, B * TSL], F32, kind="ExternalOutput")

    with tile.TileContext(nc) as tc:
        with tc.tile_pool(name="dram", bufs=1, space="DRAM") as dpool, \
             tc.tile_pool(name="const", bufs=1) as cpool, \
             tc.tile_pool(name="qkv", bufs=1) as qpool:
            # per-(batch, local-head) A2A halves: each fires as soon as that
            # head's context is complete, so the tail Wo can start on the
            # gathered half while the other half is still in flight
            ctxH_d = [[dpool.tile([NCORES * P, TSL], BF16, name=f"ctxH{b}{hl}")
                       for hl in range(HPC)] for b in range(B)]
            gouth_d = [[dpool.tile([NCORES * P, TSL], BF16, name=f"gouth{b}{hl}")
                        for hl in range(HPC)] for b in range(B)]
            bar_in = dpool.tile([8, 4], F32)
            bar_out = dpool.tile([64, 4], F32, addr_space="Shared")

            qT_sb = [qpool.tile([P, HPC, T], BF16, name=f"qT{b}") for b in range(B)]
            kT_sb = [qpool.tile([P, HPC, T], BF16, name=f"kT{b}") for b in range(B)]
            v_sb = [qpool.tile([P, NTB, DLOC], BF16, name=f"v{b}") for b in range(B)]

            cm_s = cpool.tile([P, 4 * TCH], BF16)
            wu_s = cpool.tile([P, TCH], BF16)

            # start-skew absorber: cores align here while projections run
            nc.sync.dma_start(bar_in[:], cf.ap()[0:8, 0:4])
            nc.gpsimd.collective_compute(
                "AllGather", mybir.AluOpType.bypass,
                replica_groups=[list(range(NCORES))],
                ins=[bar_in[:].opt()], outs=[bar_out[:].opt()])

            # ---------- fused projections + attention ----------
            with tc.tile_pool(name="p2", bufs=2) as pool2, \
                 tc.tile_pool(name="p2t", bufs=20) as ppool, \
                 tc.tile_pool(name="ps_s", bufs=3, space="PSUM") as ps_sp, \
                 tc.tile_pool(name="ps_acc", bufs=1, space="PSUM") as ps_accp:

                # warmup: memset a tile and run throwaway matmuls so the PE
                # is busy (and p-state ramping) while the first real loads
                # land behind the ~7us engine preamble
                nc.vector.memset(wu_s[:], 0.0)
                for _ in range(10):
                    ps_wu = ps_sp.tile([P, TCH], F32, tag="s")
                    nc.tensor.matmul(ps_wu[:], wu_s[:, 0:P], wu_s[:],
                                     start=True, stop=True)

                def attn_section(b, hl, cq):
                    """One (batch, head, 512-query-chunk) causal-attention
                    section; needs x-chunks <= cq of batch b projected.
                    Diagonal key blocks drop their fully-masked left columns
                    (widths 512/384/256/128)."""
                    nblk = 4 * cq + 4
                    q0 = cq * TCH
                    ps_ctx = ps_accp.tile([P, TCH], F32, tag="ctx")
                    # f32 accumulators (bf16 would drop the tail terms of
                    # long sums); two parallel chains halve DVE latency
                    acc0 = pool2.tile([P, TCH], F32, tag="acc0")
                    acc1 = pool2.tile([P, TCH], F32, tag="acc1")

                    def emit_scores(j):
                        vmask = j - 4 * cq
                        off = vmask * P if vmask > 0 else 0
                        ps_sc = ps_sp.tile([P, TCH], F32, tag="s")
                        nc.tensor.matmul(
                            ps_sc[:, off:], kT_sb[b][:, hl, j * P:(j + 1) * P],
                            qT_sb[b][:, hl, q0 + off:q0 + TCH],
                            start=True, stop=True)
                        pT = ppool.tile([P, TCH], BF16, tag="pT")
                        nc.scalar.activation(pT[:, off:], ps_sc[:, off:],
                                             EXP, scale=SCALE)
                        if vmask >= 0:
                            nc.vector.tensor_tensor(
                                pT[:, off:], pT[:, off:],
                                cm_s[:, vmask * TCH + off:(vmask + 1) * TCH],
                                MUL)
                        return off, pT

                    # software-pipelined: scores run two blocks ahead of
                    # the ctx matmuls (3 score banks), so ctx(j) is never
                    # at the PE queue head while exp(j) is still on Act
                    pend = [emit_scores(0)]
                    if nblk > 1:
                        pend.append(emit_scores(1))
                    for j in range(nblk):
                        off, pT = pend.pop(0)
                        if j + 2 < nblk:
                            pend.append(emit_scores(j + 2))
                        nc.tensor.matmul(
                            ps_ctx[:, off:], v_sb[b][:, j, hl * D:(hl + 1) * D],
                            pT[:, off:], start=(j == 0), stop=(j == nblk - 1),
                            skip_group_check=(off > 0))
                        # denominator: elementwise key-sum on the DVE — no
                        # PE ones-matmul at all
                        acc = acc0 if j % 2 == 0 else acc1
                        if j < 2:
                            if off > 0:
                                nc.vector.memset(acc[:, 0:off], 0.0)
                            nc.vector.tensor_copy(acc[:, off:], pT[:, off:])
                        else:
                            nc.vector.tensor_tensor(
                                acc[:, off:], acc[:, off:], pT[:, off:], ADD)
                    # release the ctx PSUM bank right away: unnormalized
                    # copy on Act; normalize SBUF->SBUF once the
                    # denominator is in (off the PE critical path)
                    ctx_u = pool2.tile([P, TCH], BF16, tag="ctx_u")
                    nc.scalar.activation(ctx_u[:], ps_ctx[:], COPY)
                    nc.vector.tensor_tensor(acc0[:], acc0[:], acc1[:], ADD)
                    # all partitions get the key-total in one gpsimd
                    # all-reduce (also kills the partition-broadcast)
                    ar = pool2.tile([P, TCH], F32, tag="ar")
                    nc.gpsimd.partition_all_reduce(
                        ar[:], acc0[:], channels=P,
                        reduce_op=bass_isa.ReduceOp.add)
                    bc_s = pool2.tile([P, TCH], F32, tag="bc_s")
                    nc.vector.reciprocal_approx_fast(bc_s[:], ar[:])
                    ctx_s = pool2.tile([P, TCH], BF16, tag="ctx")
                    nc.vector.tensor_tensor(ctx_s[:], ctx_u[:], bc_s[:], MUL)
                    nc.sync.dma_start(
                        ctxH_d[b][hl]
                        .rearrange("(r p) n -> p r n", p=P)[:, 2 * cq:2 * cq + 2],
                        ctx_s.rearrange("p (r n) -> p r n", r=2))

                projpools = tc.tile_pool(name="p1w", bufs=1), \
                    tc.tile_pool(name="p1cf", bufs=1), \
                    tc.tile_pool(name="p1x", bufs=2), \
                    tc.tile_pool(name="p1", bufs=2), \
                    tc.tile_pool(name="ps1", bufs=3, space="PSUM"), \
                    tc.tile_pool(name="ps1v", bufs=1, space="PSUM")
                wpool = projpools[0].__enter__()
                cfpool = projpools[1].__enter__()
                xpool = projpools[2].__enter__()
                pool = projpools[3].__enter__()
                ps1 = projpools[4].__enter__()
                ps1v = projpools[5].__enter__()
                wsb = wpool.tile([P, NKB, 2 * DLOC], BF16)
                wv_s = wpool.tile([P, NKB, DLOC], BF16)
                cf_s = cfpool.tile([P, T], F32)
                sf_s = cfpool.tile([P, T], F32)

                for ip in range(BT // XCH):     # 4 chunk-pairs
                    bb, icp = ip // 2, ip % 2
                    xt_t = xpool.tile([P, NKB, XCH], BF16, tag="xt")

                    def xt_load(kp, eng):
                        c0 = (ip * NKB + 2 * kp) * XCH
                        eng.dma_start(
                            xt_t[:, 2 * kp:2 * kp + 2],
                            xt2.ap()[:, c0:c0 + 2 * XCH]
                            .rearrange("p (k n) -> p k n", k=2))

                    # ip0/ip1 loads are wait-free (fresh tiles), so they
                    # may ride the scalar ring too — no exp runs before
                    # them that they could block; ip2/ip3 reuse xt buffers
                    # (WAR waits) and must stay on sync
                    if ip == 0:
                        # weight/x kb-pairs interleaved in consumption
                        # order across both rings; RoPE tables, V weights
                        # and mask tiles slot in at their first-use
                        # deadlines
                        for kp in range(NKB // 2):
                            c0 = 2 * kp * 2 * DLOC
                            nc.scalar.dma_start(
                                wsb[:, 2 * kp:2 * kp + 2],
                                wqk.ap()[:, c0:c0 + 4 * DLOC]
                                .rearrange("p (k n) -> p k n", k=2))
                            xt_load(kp, nc.sync)
                            if kp == 2:
                                nc.sync.dma_start(cf_s[:], cf.ap())
                            if kp == 4:
                                nc.scalar.dma_start(sf_s[:], sf.ap())
                            if kp == 6:
                                nc.scalar.dma_start(
                                    wv_s[:, 0:8],
                                    wvp.ap()[:, 0:8 * DLOC]
                                    .rearrange("p (k n) -> p k n", k=8))
                        nc.scalar.dma_start(
                            wv_s[:, 8:16],
                            wvp.ap()[:, 8 * DLOC:16 * DLOC]
                            .rearrange("p (k n) -> p k n", k=8))
                        nc.scalar.dma_start(cm_s[:], cm.ap())
                    elif ip == 1:
                        for kp in range(NKB // 2):
                            xt_load(kp, nc.sync if kp % 2 == 0 else nc.scalar)
                    else:
                        for kp in range(NKB // 2):
                            xt_load(kp, nc.sync)
                    # two query-chunk columns per stationary pass
                    for w0, dst in ((0, qT_sb), (DLOC, kT_sb)):
                        for m in range(HPC):
                            psa = ps1.tile([P, TCH], F32, tag="qk")
                            psb = ps1.tile([P, TCH], F32, tag="qk")
                            for kb in range(NKB):
                                st, sp = (kb == 0), (kb == NKB - 1)
                                w_blk = wsb[:, kb, w0 + m * P:w0 + (m + 1) * P]
                                nc.tensor.matmul(psa[:], w_blk,
                                                 xt_t[:, kb, 0:TCH],
                                                 start=st, stop=sp)
                                nc.tensor.matmul(psb[:], w_blk,
                                                 xt_t[:, kb, TCH:XCH],
                                                 start=st, stop=sp)
                            for half, ps in ((0, psa), (1, psb)):
                                ic = 2 * icp + half
                                c0 = ic * TCH
                                cs = cf_s[:, c0:c0 + TCH]
                                sn = sf_s[:, c0:c0 + TCH]
                                tmp = pool.tile([P, TCH], F32, tag="tmp")
                                tmp2 = pool.tile([P, TCH], F32, tag="tmp2")
                                nc.vector.tensor_tensor(tmp[0:64], ps[64:128],
                                                        sn[0:64], MUL)
                                nc.vector.tensor_tensor(tmp[64:128], ps[0:64],
                                                        sn[64:128], MUL)
                                nc.vector.tensor_tensor(tmp2[:], ps[:], cs, MUL)
                                nc.vector.tensor_tensor(
                                    dst[bb][:, m, c0:c0 + TCH],
                                    tmp2[:], tmp[:], ADD)
                    for tb in range(XCH // P):
                        psv = ps1v.tile([P, DLOC], F32, tag="v")
                        for kb in range(NKB):
                            nc.tensor.matmul(
                                psv[:], xt_t[:, kb, tb * P:(tb + 1) * P],
                                wv_s[:, kb],
                                start=(kb == 0), stop=(kb == NKB - 1))
                        nc.scalar.activation(
                            v_sb[bb][:, icp * (XCH // P) + tb, :], psv[:],
                            COPY)
                    # attention sections whose query chunks now exist; all
                    # of b1's sections are deferred past the projection
                    # pools so they interleave with Wo-b0 instead (at
                    # ip2/ip3's end there is no projection fill left)
                    if ip == 0:
                        for cq in (0, 1):
                            for hl in range(HPC):
                                attn_section(bb, hl, cq)
                    elif ip == 1:
                        for hl in range(HPC):
                            for cq in (2, 3):
                                attn_section(bb, hl, cq)
                            nc.gpsimd.collective_compute(
                                "AllToAll", mybir.AluOpType.bypass,
                                replica_groups=[list(range(NCORES))],
                                ins=[ctxH_d[0][hl][:].opt()],
                                outs=[gouth_d[0][hl][:].opt()])

                for p in reversed(projpools):
                    p.__exit__(None, None, None)

                # ---------- tail: last sections + output projection ----------
                with tc.tile_pool(name="p3w", bufs=1) as wpool3, \
                     tc.tile_pool(name="p3", bufs=2) as pool3, \
                     tc.tile_pool(name="ps3", bufs=4, space="PSUM") as ps3:
                    # wo_s[:, e] holds out-columns [e*256, (e+1)*256) for
                    # all kbs — m-tile m lives in eighth m//2
                    wo_s = wpool3.tile([P, 8, NKB, DLOC], BF16)
                    g_t = [wpool3.tile([P, NKB, TSL], BF16, name=f"g{b}")
                           for b in range(B)]

                    def gather_load(b, hl, engs):
                        # per-r 2D gathers (128 descriptors each); gouth
                        # block r holds global head 2r+hl -> kb slot 2r+hl
                        for r in range(NCORES):
                            engs[r % len(engs)].dma_start(
                                g_t[b][:, 2 * r + hl],
                                gouth_d[b][hl]
                                .rearrange("(r p) n -> r p n", p=P)[r])

                    # Wo streams in column-eighths (contiguous 2D slices of
                    # the host-packed woG) so m-tiles unblock progressively;
                    # b0 gathers (A2As fired back at ip1) slot in after the
                    # first two eighths — everything lands just before its
                    # first consumer
                    def wo_load(e):
                        nc.sync.dma_start(
                            wo_s[:, e],
                            woG.ap()[:, e * NKB * DLOC:(e + 1) * NKB * DLOC]
                            .rearrange("p (k n) -> p k n", k=NKB))

                    wo_load(0)
                    wo_load(1)
                    for hl in range(HPC):
                        gather_load(0, hl, (nc.sync,))
                    for e in range(2, 8):
                        wo_load(e)

                    def wo_b0(mlist, alt=False):
                        for m in mlist:
                            pso = ps3.tile([P, TSL], F32, tag="o")
                            for kb in range(NKB):
                                nc.tensor.matmul(
                                    pso[:],
                                    wo_s[:, m // 2, kb,
                                         (m % 2) * P:(m % 2 + 1) * P],
                                    g_t[0][:, kb],
                                    start=(kb == 0), stop=(kb == NKB - 1))
                            o_s = pool3.tile([P, TSL], F32, tag="o_s")
                            nc.vector.tensor_copy(o_s[:], pso[:])
                            # while exps still run, tail DMA rides sync so
                            # the scalar queue stays pure Act compute; once
                            # sections are done (alt=True) writes alternate
                            # rings so the final drain halves
                            eng = nc.scalar if alt and m % 2 else nc.sync
                            eng.dma_start(
                                outT.ap()[m * P:(m + 1) * P, 0:TSL], o_s[:])

                    # hl=1 first: its A2A feeds the first half of the final
                    # Wo-b1 accumulation, so firing it early hides the
                    # collective + peer skew under Wo-b0; a couple of Wo-b0
                    # m-tiles sit between sections as PE fill, but both
                    # A2As stay early so the b1 accumulation never waits
                    attn_section(1, 1, 0)
                    attn_section(1, 1, 1)
                    attn_section(1, 1, 2)
                    wo_b0([0, 1])
                    attn_section(1, 1, 3)
                    nc.gpsimd.collective_compute(
                        "AllToAll", mybir.AluOpType.bypass,
                        replica_groups=[list(range(NCORES))],
                        ins=[ctxH_d[1][1][:].opt()],
                        outs=[gouth_d[1][1][:].opt()])
                    attn_section(1, 0, 0)
                    wo_b0([2, 3])
                    attn_section(1, 0, 1)
                    attn_section(1, 0, 2)
                    wo_b0([4, 5])
                    attn_section(1, 0, 3)
                    nc.gpsimd.collective_compute(
                        "AllToAll", mybir.AluOpType.bypass,
                        replica_groups=[list(range(NCORES))],
                        ins=[ctxH_d[1][0][:].opt()],
                        outs=[gouth_d[1][0][:].opt()])
                    # gathers only now: every ctx scatter and exp is
                    # already queued ahead of them, so their parked A2A
                    # waits can no longer delay a collective's input; the
                    # last gather splits across both rings to halve its
                    # post-A2A issue latency
                    gather_load(1, 1, (nc.sync,))
                    gather_load(1, 0, (nc.sync, nc.scalar))
                    wo_b0([6, 7, 8, 9, 10, 11, 12, 13, 14, 15], alt=True)
                    # Wo-b1: one PSUM pass per m-tile — odd kbs (early A2A)
                    # first, even kbs accumulate on top once their gather
                    # lands; no SBUF staging, no DVE adds
                    for m in range(DM // P):
                        psA = ps3.tile([P, TSL], F32, tag="o")
                        for i in range(NCORES):
                            nc.tensor.matmul(
                                psA[:],
                                wo_s[:, m // 2, 2 * i + 1,
                                     (m % 2) * P:(m % 2 + 1) * P],
                                g_t[1][:, 2 * i + 1],
                                start=(i == 0), stop=False)
                        for i in range(NCORES):
                            nc.tensor.matmul(
                                psA[:],
                                wo_s[:, m // 2, 2 * i,
                                     (m % 2) * P:(m % 2 + 1) * P],
                                g_t[1][:, 2 * i],
                                start=False, stop=(i == NCORES - 1))
                        o_s = pool3.tile([P, TSL], F32, tag="o_s")
                        nc.vector.tensor_copy(o_s[:], psA[:])
                        eng = nc.sync if m % 2 == 0 else nc.scalar
                        eng.dma_start(
                            outT.ap()[m * P:(m + 1) * P, TSL:2 * TSL], o_s[:])

    nc.compile()
    return nc


def _prep_inputs(x, cos, sin, Wq, Wk, Wv, Wo):
    x = np.asarray(x, dtype=np.float32)
    cos = np.asarray(cos, dtype=np.float32)
    sin = np.asarray(sin, dtype=np.float32)
    xt = np.ascontiguousarray(x.reshape(BT, DM).T).astype(ml_dtypes.bfloat16)
    # xt2[p, ip*16K + kb*1K + n] = xt[kb*128+p, ip*1024+n]
    xt2 = np.ascontiguousarray(
        xt.reshape(NKB, P, 4, XCH).transpose(1, 2, 0, 3).reshape(P, -1))
    cf = np.empty((P, T), np.float32)
    cf[:64] = cos.T
    cf[64:] = cos.T
    sf = np.empty((P, T), np.float32)
    sf[:64] = -sin.T
    sf[64:] = sin.T
    qq = np.arange(TCH, dtype=np.int64)[None, :]
    rr = np.arange(P, dtype=np.int64)[:, None]
    cm = np.concatenate(
        [(qq >= v * P + rr).astype(np.float32) for v in range(TCH // P)],
        axis=1).astype(ml_dtypes.bfloat16)
    # woG[p, e*4K + kb*256 + c] = Wo[kb*128+p, e*256+c]
    wo16 = np.asarray(Wo, np.float32).astype(ml_dtypes.bfloat16)
    woG = np.ascontiguousarray(
        wo16.reshape(NKB, P, 8, DLOC).transpose(1, 2, 0, 3).reshape(P, -1))
    wq16 = np.asarray(Wq, np.float32).astype(ml_dtypes.bfloat16)
    wk16 = np.asarray(Wk, np.float32).astype(ml_dtypes.bfloat16)
    wv16 = np.asarray(Wv, np.float32).astype(ml_dtypes.bfloat16)
    in_maps = []
    for c in range(NCORES):
        sl = slice(c * DLOC, (c + 1) * DLOC)
        # wqk[p, kb*512 + j]: j in [0,256) wq | [256,512) wk
        wqk = np.ascontiguousarray(
            np.concatenate(
                [wq16[:, sl].reshape(NKB, P, DLOC),
                 wk16[:, sl].reshape(NKB, P, DLOC)],
                axis=2).transpose(1, 0, 2).reshape(P, -1))
        wvp = np.ascontiguousarray(
            wv16[:, sl].reshape(NKB, P, DLOC).transpose(1, 0, 2)
            .reshape(P, -1))
        in_maps.append({
            "xt2": xt2, "cf": cf, "sf": sf, "cm": cm,
            "wqk": wqk, "wvp": wvp, "woG": woG,
        })
    return in_maps


def run(x, mask, cos, sin, Wq, Wk, Wv, Wo, trace=False, trace_cores=None):
    global _nc_cache
    if _nc_cache is None:
        _nc_cache = _build()
    in_maps = _prep_inputs(x, cos, sin, Wq, Wk, Wv, Wo)
    kwargs = {"trace_cores": trace_cores} if trace_cores else {}
    res = bass_utils.run_bass_kernel_spmd(
        _nc_cache, in_maps, core_ids=list(range(NCORES)), trace=trace, **kwargs)
    out = np.empty((B, T, DM), np.float32)
    for c in range(NCORES):
        o = res.results[c]["out"]  # [DM, B*TSL]
        for b in range(B):
            out[b, c * TSL:(c + 1) * TSL, :] = o[:, b * TSL:(b + 1) * TSL].T
    return out, res


def kernel(x, mask, cos, sin, Wq, Wk, Wv, Wo):
    out, _ = run(x, mask, cos, sin, Wq, Wk, Wv, Wo, trace=False)
    return out
